# revision 1
# baseline (speedup 1.0000x reference)
"""Trainium2 Bass kernel for the expert-choice MoE layer (nn_MoELayer_18451179504170).

Strategy: expert-parallel across 8 NeuronCores (2 experts/core). Each core gets
the full hidden_states, the replicated router, and its 2 experts' weights.
On device (per core):
  1. Router: PE-transpose X tiles -> X^T, mm X^T @ R1 -> silu -> @ R2 giving
     logit rows [e_loc, tok].
  2. Top-256 per (batch, expert): gpsimd.kth_largest gives the 257th-largest
     logit tau; mask = (l > tau); gpsimd.sparse_gather compacts winner token
     ids (and exp(l - tau) weights) into the wrap-16 layout that
     dma_gather / dma_scatter_add consume natively.
  3. Dispatch: dma_gather pulls the 256 selected rows per (b,e) from DRAM;
     PE-transposes them to Xg^T.
  4. Experts (SwiGLU): fp32 matmuls; W1/W2 are stationary (host-pre-tiled),
     gate/value in PSUM; h = silu(gate)*value stays in [dff, tok] layout so the
     second matmul (W3) needs no transposes and directly yields token-major
     rows; per-token gate weights applied as per-partition scales during the
     PSUM->SBUF copy.
  5. Combine: dma_scatter_add into this core's (pre-zeroed) partial output.
Host: sums the 8 partial outputs.
"""

import os
import sys

for _p in ("/opt/trn_rl_repo", "/root/.axon_site/_ro/trn_rl_repo"):
    if os.path.isdir(_p) and _p not in sys.path:
        sys.path.insert(0, _p)

import numpy as np

import concourse.bass as bass  # noqa: F401
import concourse.mybir as mybir
from concourse import bacc
from concourse.tile import TileContext
from concourse.bass_utils import run_bass_kernel_spmd

F32 = mybir.dt.float32
F32R = mybir.dt.float32r
AF = mybir.ActivationFunctionType
OP = mybir.AluOpType

B, S, D = 4, 2048, 1024
E, DFF = 16, 2048
CAP = 256
RH = 128          # router hidden
EL = 2            # experts per core
NCORES = 8
NTOK = B * S      # 8192
NT = NTOK // 128  # 64 token tiles
NG = NT // 4      # 16 groups of 4 tiles

DEBUG = bool(int(os.environ.get("MOE_KERNEL_DEBUG", "0")))


def _build_program():
    nc = bacc.Bacc(None, target_bir_lowering=False)

    hs = nc.dram_tensor("hs", [NTOK, D], F32, kind="ExternalInput")
    hstt = nc.dram_tensor("hstt", [NG, 8, 128, 512], F32, kind="ExternalInput")
    r1t = nc.dram_tensor("r1t", [128, 8 * RH], F32, kind="ExternalInput")
    r2c = nc.dram_tensor("r2c", [RH, EL], F32, kind="ExternalInput")
    w1t = nc.dram_tensor("w1t", [EL, 16, 128, 1024], F32R, kind="ExternalInput")
    w2t = nc.dram_tensor("w2t", [EL, 16, 128, 1024], F32R, kind="ExternalInput")
    w3c = nc.dram_tensor("w3c", [EL, DFF, D], F32R, kind="ExternalInput")
    ident = nc.dram_tensor("ident", [128, 128], F32, kind="ExternalInput")
    rep16 = nc.dram_tensor("rep16", [16, 128], F32, kind="ExternalInput")
    ones_1_16 = nc.dram_tensor("ones_1_16", [1, 16], F32, kind="ExternalInput")
    ones16_1 = nc.dram_tensor("ones16_1", [16, 1], F32, kind="ExternalInput")
    iota_w = nc.dram_tensor("iota_w", [16, 1024], F32, kind="ExternalInput")

    outp = nc.dram_tensor("outp", [NTOK, D], F32, kind="ExternalOutput")
    if DEBUG:
        d_lT = nc.dram_tensor("d_lT", [EL, NTOK], F32, kind="ExternalOutput")
        d_tau = nc.dram_tensor("d_tau", [1, 16], F32, kind="ExternalOutput")
        d_nf = nc.dram_tensor("d_nf", [1, 16], mybir.dt.uint32, kind="ExternalOutput")
        d_idx = nc.dram_tensor("d_idx", [128, 128], mybir.dt.int16, kind="ExternalOutput")
        d_wpp = nc.dram_tensor("d_wpp", [128, 16], F32, kind="ExternalOutput")

    with TileContext(nc) as tc:
        with (
            tc.tile_pool(name="const", bufs=1) as cpool,
            tc.tile_pool(name="persist", bufs=1) as ppool,
        ):
            c_ident = cpool.tile([128, 128], F32)
            nc.sync.dma_start(out=c_ident, in_=ident[:, :])
            c_rep16 = cpool.tile([16, 128], F32)
            nc.sync.dma_start(out=c_rep16, in_=rep16[:, :])
            c_o116 = cpool.tile([1, 16], F32)
            nc.sync.dma_start(out=c_o116, in_=ones_1_16[:, :])
            c_o161 = cpool.tile([16, 1], F32)
            nc.sync.dma_start(out=c_o161, in_=ones16_1[:, :])
            c_iota = cpool.tile([16, 1024], F32)
            nc.sync.dma_start(out=c_iota, in_=iota_w[:, :])
            c_r1t = cpool.tile([128, 8 * RH], F32)
            nc.sync.dma_start(out=c_r1t, in_=r1t[:, :])
            c_r2c = cpool.tile([RH, EL], F32)
            nc.sync.dma_start(out=c_r2c, in_=r2c[:, :])

            p_idx16 = ppool.tile([128, 128], mybir.dt.int16)
            p_wpp = ppool.tile([128, 16], F32)
            p_cand2 = ppool.tile([16, 1024], F32)
            p_wself = ppool.tile([16, 160], F32)
            p_nfw = ppool.tile([1, 16], mybir.dt.uint32)

            # ---------------- Phase R + T: router and top-k ----------------
            with tc.tile_pool(name="rt_sb", bufs=1) as rtpool:
                p_lTb = []  # [ei][b] -> [1, 2048] logit row tiles
                for _ei in range(EL):
                    row = []
                    for _b in range(B):
                        lt_t = rtpool.tile([1, S], F32, tag=f"lt{_ei}{_b}")
                        row.append(lt_t)
                    p_lTb.append(row)

                with tc.tile_pool(name="r_pres", bufs=1) as prespool:
                    preS = prespool.tile([128, NTOK], F32)  # silu(X@R1)^T [rh, tok]
                    with (
                        tc.tile_pool(name="r_xts", bufs=8) as xtspool,
                        tc.tile_pool(name="r_pspre", bufs=2, space="PSUM") as pspre,
                        tc.tile_pool(name="r_pslg", bufs=2, space="PSUM") as pslg,
                    ):
                        for g in range(NG):
                            xk_tiles = []
                            for k in range(8):
                                xk = xtspool.tile([128, 512], F32, tag="xts")
                                nc.sync.dma_start(out=xk, in_=hstt[g, k, :, :])
                                xk_tiles.append(xk)
                            ps_pre = pspre.tile([128, 512], F32, tag="pspre")
                            for k in range(8):
                                nc.tensor.matmul(
                                    ps_pre, c_r1t[:, 128 * k:128 * (k + 1)],
                                    xk_tiles[k], start=(k == 0), stop=(k == 7))
                            nc.scalar.activation(
                                preS[:, 512 * g:512 * (g + 1)], ps_pre, AF.Silu)
                            b, gb = g // 4, g % 4
                            for ei in range(EL):
                                ps_lg = pslg.tile([1, 512], F32, tag="pslg")
                                nc.tensor.matmul(ps_lg, c_r2c[:, ei:ei + 1],
                                                 preS[:, 512 * g:512 * (g + 1)],
                                                 start=True, stop=True)
                                nc.scalar.activation(
                                    p_lTb[ei][b][:, 512 * gb:512 * (gb + 1)],
                                    ps_lg, AF.Copy)

                # ---------------- top-k ----------------
                with (
                    tc.tile_pool(name="t_sb", bufs=1) as tpool,
                    tc.tile_pool(name="t_ps", bufs=2, space="PSUM") as tps,
                ):
                    # l_w128[q][p, j] = lT[ei][b][16p + j]  (any order is fine
                    # for the quantile; same source bits as the masks below).
                    # Per-q tiles so each kth_largest only waits on its batch.
                    tau8 = tpool.tile([1, 16], F32)
                    qq = 1.0 - 255.5 / 2047.0
                    for ei in range(EL):
                        for b in range(B):
                            q = 4 * ei + b
                            lwq = tpool.tile([128, 16], F32, tag=f"lw{q}")
                            srcap = p_lTb[ei][b][0:1, :].rearrange(
                                "o (p j) -> o p j", p=128, j=16)
                            nc.gpsimd.dma_start(out=lwq, in_=srcap)
                            nc.gpsimd.kth_largest(tau8[0:1, 2 * q:2 * q + 2],
                                                  lwq, n_per_lane=16, k=300,
                                                  quantile=qq)
                    # col 2q+1 = exact 257th-largest value; broadcast to [16, 8]
                    tau_odd = tau8[:, :].rearrange("o (q c) -> o c q", c=2)[:, 1, :]
                    ps_tau16 = tps.tile([16, 8], F32, tag="tps")
                    nc.tensor.matmul(ps_tau16, c_o116, tau_odd, start=True, stop=True)
                    tau16 = tpool.tile([16, 8], F32)
                    nc.scalar.activation(tau16, ps_tau16, AF.Copy)

                    # wrap 16: l_wrap[p, 128*q + f] = lT[ei, 2048*b + 128*p + f]
                    l_wrap = tpool.tile([16, 1024], F32)
                    for ei in range(EL):
                        for b in range(B):
                            srcap = p_lTb[ei][b][0:1, :].rearrange(
                                "o (p f) -> o p f", p=16, f=128)
                            q = 4 * ei + b
                            nc.gpsimd.dma_start(
                                out=l_wrap[:, 128 * q:128 * (q + 1)], in_=srcap)

                    t16b = tau16[:, :].to_broadcast([16, 8, 128])
                    lw_r = l_wrap[:, :].rearrange("p (q f) -> p q f", q=8)
                    cmp = tpool.tile([16, 1024], mybir.dt.uint8)
                    cmp_r = cmp[:, :].rearrange("p (q f) -> p q f", q=8)
                    nc.vector.tensor_tensor(cmp_r, lw_r, t16b, OP.is_gt)
                    cand = tpool.tile([16, 1024], F32)
                    nc.vector.memset(cand, -1.0)
                    nc.vector.copy_predicated(cand, cmp, c_iota)
                    esub = tpool.tile([16, 1024], F32)
                    esub_r = esub[:, :].rearrange("p (q f) -> p q f", q=8)
                    nc.vector.tensor_tensor(esub_r, lw_r, t16b, OP.subtract)
                    eexp = tpool.tile([16, 1024], F32)
                    nc.scalar.activation(eexp, esub, AF.Exp)
                    nc.vector.memset(p_cand2, -1.0)
                    nc.vector.copy_predicated(p_cand2, cmp, eexp)

                    idxf = tpool.tile([16, 160], F32)
                    nf = tpool.tile([1, 16], mybir.dt.uint32)
                    for q in range(8):
                        nc.gpsimd.sparse_gather(idxf[:, 20 * q:20 * q + 20],
                                                cand[:, 128 * q:128 * (q + 1)],
                                                num_found=nf[0:1, q:q + 1])

                    # replicate idx to 128 partitions, cast int16 (gathers wait on this)
                    idx_r = idxf[:, :].rearrange("p (q x) -> p q x", q=8)[:, :, 0:16]
                    ps_idx = tps.tile([128, 128], F32, tag="tpsbig")
                    nc.tensor.matmul(ps_idx, c_rep16, idx_r, start=True, stop=True)
                    nc.vector.tensor_copy(p_idx16, ps_idx)


                    if DEBUG:
                        for _ei in range(EL):
                            for _b in range(B):
                                nc.sync.dma_start(
                                    out=d_lT[_ei:_ei + 1, S * _b:S * (_b + 1)],
                                    in_=p_lTb[_ei][_b])
                        nc.sync.dma_start(out=d_tau[:, :], in_=tau8)
                        nc.sync.dma_start(out=d_nf[:, :], in_=nf)
                        nc.sync.dma_start(out=d_idx[:, :], in_=p_idx16)
                        nc.sync.dma_start(out=d_wpp[:, :], in_=p_wpp)

            # ---------------- Experts ----------------
            for ei in range(EL):
                with (
                    tc.tile_pool(name=f"e{ei}_xgt", bufs=1) as xgtpool,
                    tc.tile_pool(name=f"e{ei}_xg", bufs=2) as xgpool,
                    tc.tile_pool(name=f"e{ei}_h", bufs=16) as hpool,
                    tc.tile_pool(name=f"e{ei}_wm", bufs=4) as wmpool,
                    tc.tile_pool(name=f"e{ei}_w3", bufs=3) as w3pool,
                    tc.tile_pool(name=f"e{ei}_orow", bufs=4) as orowpool,
                ):
                    xgt = xgtpool.tile([128, 8 * 1024], F32R)  # [D-chunk part, k*1024 + tok]
                    xgt_r = xgt[:, :].rearrange("p (k t) -> p k t", k=8)
                    with tc.tile_pool(name=f"e{ei}_psxt", bufs=2, space="PSUM") as psxt2:
                        for b in range(B):
                            q = 4 * ei + b
                            xg = xgpool.tile([128, 2, 1024], F32, tag="xg")
                            nc.gpsimd.dma_gather(
                                xg, hs[2048 * b:2048 * (b + 1), :],
                                p_idx16[:, 16 * q:16 * (q + 1)],
                                num_idxs=CAP, num_idxs_reg=CAP, elem_size=D)
                            for s in range(2):
                                ps_t = psxt2.tile([128, 1024], F32, tag="psxt2")
                                for k in range(8):
                                    nc.tensor.transpose(
                                        ps_t[:, 128 * k:128 * (k + 1)],
                                        xg[:, s, 128 * k:128 * (k + 1)], c_ident)
                                dst = xgt_r[:, :, 256 * b + 128 * s: 256 * b + 128 * (s + 1)]
                                src = ps_t[:, :].rearrange("p (k t) -> p k t", k=8)
                                if s == 0:
                                    nc.vector.tensor_copy(dst, src)
                                else:
                                    nc.scalar.activation(dst, src, AF.Copy)

                    if ei == 0:
                        # deferred gating-weight chain: runs after e0's gathers
                        # are queued on gpsimd; results only needed at mm3 time
                        with tc.tile_pool(name="wq_ps", bufs=2, space="PSUM") as wps:
                            for q in range(8):
                                nc.gpsimd.sparse_gather(
                                    p_wself[:, 20 * q:20 * q + 20],
                                    p_cand2[:, 128 * q:128 * (q + 1)],
                                    num_found=p_nfw[0:1, q:q + 1])
                            wsel_r = p_wself[:, :].rearrange(
                                "p (q x) -> p q x", q=8)[:, :, 0:16]
                            ps_sum = wps.tile([1, 128], F32, tag="wps")
                            nc.tensor.matmul(ps_sum, c_o161, wsel_r,
                                             start=True, stop=True)
                            sums = ppool.tile([1, 8], F32, tag="sums")
                            nc.vector.tensor_reduce(
                                sums, ps_sum[:, :].rearrange("p (q x) -> p q x", q=8),
                                mybir.AxisListType.X, OP.add)
                            nc.vector.tensor_scalar_add(sums, sums, 1e-9)
                            rec = ppool.tile([1, 8], F32, tag="rec")
                            nc.vector.reciprocal(rec, sums)
                            ps_rec16 = wps.tile([16, 8], F32, tag="wps")
                            nc.tensor.matmul(ps_rec16, c_o116, rec,
                                             start=True, stop=True)
                            rec16 = ppool.tile([16, 8], F32, tag="rec16")
                            nc.scalar.activation(rec16, ps_rec16, AF.Copy)
                            wnorm = ppool.tile([16, 160], F32, tag="wnorm")
                            wn_r = wnorm[:, :].rearrange("p (q x) -> p q x", q=8)
                            ws_r = p_wself[:, :].rearrange("p (q x) -> p q x", q=8)
                            nc.vector.tensor_tensor(
                                wn_r, ws_r, rec16[:, :].to_broadcast([16, 8, 20]),
                                OP.mult)
                            wn_sx = wnorm[:, :].rearrange("p (q x) -> p x q", q=8)
                            for g in range(8):
                                for s in range(2):
                                    dstap = p_wpp[16 * g:16 * (g + 1), :].rearrange(
                                        "p (q s) -> p s q", q=8, s=2)[:, s, :]
                                    nc.gpsimd.dma_start(out=dstap,
                                                        in_=wn_sx[:, 8 * s + g, :])

                    # mm1/mm2 + swiglu -> h tiles
                    h_tiles = []
                    with tc.tile_pool(name=f"e{ei}_gv", bufs=3, space="PSUM") as psgv:
                        for m in range(16):
                            w1m = wmpool.tile([128, 1024], F32R, tag="wm")
                            nc.sync.dma_start(out=w1m, in_=w1t[ei, m, :, :])
                            w2m = wmpool.tile([128, 1024], F32R, tag="wm")
                            nc.sync.dma_start(out=w2m, in_=w2t[ei, m, :, :])
                            ps_g = psgv.tile([128, 1024], F32, tag="gv")
                            ps_v = psgv.tile([128, 1024], F32, tag="gv")
                            for k in range(8):
                                for hh in range(2):
                                    nc.tensor.matmul(
                                        ps_g[:, 512 * hh:512 * (hh + 1)],
                                        w1m[:, 128 * k:128 * (k + 1)],
                                        xgt[:, 1024 * k + 512 * hh: 1024 * k + 512 * (hh + 1)],
                                        start=(k == 0), stop=(k == 7))
                            for k in range(8):
                                for hh in range(2):
                                    nc.tensor.matmul(
                                        ps_v[:, 512 * hh:512 * (hh + 1)],
                                        w2m[:, 128 * k:128 * (k + 1)],
                                        xgt[:, 1024 * k + 512 * hh: 1024 * k + 512 * (hh + 1)],
                                        start=(k == 0), stop=(k == 7))
                            hm = hpool.tile([128, 1024], F32R, tag="h")
                            nc.scalar.activation(hm, ps_g, AF.Silu)
                            nc.vector.tensor_mul(hm, hm, ps_v)
                            h_tiles.append(hm)

                    # mm3: out rows, 8 psum groups, W3 streamed twice (dh outer)
                    orows = []
                    for _b in range(B):
                        orow_t = orowpool.tile([128, 2048], F32, tag="or")
                        orows.append(orow_t)
                    with tc.tile_pool(name=f"e{ei}_pso", bufs=8, space="PSUM") as pso:
                        for dh in range(2):
                            ps_os = []
                            for _bs in range(8):
                                ps_o = pso.tile([128, 512], F32, tag="pso")
                                ps_os.append(ps_o)
                            for k in range(16):
                                w3k = w3pool.tile([128, 512], F32R, tag="w3")
                                nc.sync.dma_start(
                                    out=w3k,
                                    in_=w3c[ei, 128 * k:128 * (k + 1),
                                            512 * dh:512 * (dh + 1)])
                                for bs in range(8):
                                    b, s = bs // 2, bs % 2
                                    nc.tensor.matmul(
                                        ps_os[bs],
                                        h_tiles[k][:, 256 * b + 128 * s: 256 * b + 128 * (s + 1)],
                                        w3k,
                                        start=(k == 0), stop=(k == 15))
                            for bs in range(8):
                                b, s = bs // 2, bs % 2
                                col = 8 * ei + 2 * b + s
                                dst = orows[b][:, 1024 * s + 512 * dh: 1024 * s + 512 * (dh + 1)]
                                if bs % 2 == 0:
                                    nc.vector.tensor_scalar(
                                        dst, ps_os[bs], p_wpp[:, col:col + 1], None,
                                        op0=OP.mult)
                                else:
                                    nc.scalar.activation(
                                        dst, ps_os[bs], AF.Copy,
                                        scale=p_wpp[:, col:col + 1])

                    for b in range(B):
                        q = 4 * ei + b
                        nc.gpsimd.dma_scatter_add(
                            outp[2048 * b:2048 * (b + 1), :],
                            orows[b][:, :].rearrange("p (s t) -> p s t", s=2),
                            p_idx16[:, 16 * q:16 * (q + 1)],
                            num_idxs=CAP, num_idxs_reg=CAP, elem_size=D)

    nc.finalize()
    return nc


_PROGRAM = None


def _get_program():
    global _PROGRAM
    if _PROGRAM is None:
        _PROGRAM = _build_program()
    return _PROGRAM


def _host_inputs(hidden_states, router_w1, router_w2, w1, w2, w3):
    """Builds per-core in_maps (host-side slicing / retiling)."""
    hs = np.ascontiguousarray(hidden_states.reshape(NTOK, D)).astype(np.float32)
    r1t = np.ascontiguousarray(
        np.asarray(router_w1, np.float32).reshape(8, 128, RH).transpose(1, 0, 2)
    ).reshape(128, 8 * RH)
    ident = np.eye(128, dtype=np.float32)
    rep16 = np.zeros((16, 128), np.float32)
    for m in range(128):
        rep16[m % 16, m] = 1.0
    ones_1_16 = np.ones((1, 16), np.float32)
    ones16_1 = np.ones((16, 1), np.float32)
    iota_w = np.tile(
        (np.arange(16, dtype=np.float32)[:, None] * 128.0
         + np.arange(128, dtype=np.float32)[None, :]), (1, 8))

    def tile_w(we):  # [D, DFF] -> [16, 128, 1024]: tile[m][p][k*128+c] = we[128k+p, 128m+c]
        return np.ascontiguousarray(
            we.reshape(8, 128, 16, 128).transpose(2, 1, 0, 3)).reshape(16, 128, 1024)

    w1 = np.asarray(w1, np.float32)
    w2 = np.asarray(w2, np.float32)
    w3 = np.asarray(w3, np.float32)
    r2 = np.asarray(router_w2, np.float32)

    hsT = np.ascontiguousarray(hs.T)  # [D, NTOK]
    hstt = np.ascontiguousarray(
        hsT.reshape(8, 128, NG, 512).transpose(2, 0, 1, 3))  # [g, k, 128, 512]
    in_maps = []
    for c in range(NCORES):
        e0 = EL * c
        w1c = np.stack([tile_w(w1[e0 + j]) for j in range(EL)])
        w2c = np.stack([tile_w(w2[e0 + j]) for j in range(EL)])
        w3cc = np.ascontiguousarray(w3[e0:e0 + EL])
        in_maps.append({
            "hs": hs, "hstt": hstt,
            "r1t": r1t,
            "r2c": np.ascontiguousarray(r2[:, e0:e0 + EL]),
            "w1t": w1c, "w2t": w2c, "w3c": w3cc,
            "ident": ident, "rep16": rep16,
            "ones_1_16": ones_1_16, "ones16_1": ones16_1, "iota_w": iota_w,
        })
    return in_maps


_LAST_RESULTS = None  # for test introspection


def kernel(hidden_states, router_w1, router_w2, w1, w2, w3):
    global _LAST_RESULTS
    nc = _get_program()
    in_maps = _host_inputs(hidden_states, router_w1, router_w2, w1, w2, w3)
    trace = bool(int(os.environ.get("MOE_KERNEL_TRACE", "0")))
    res = run_bass_kernel_spmd(nc, in_maps, core_ids=list(range(NCORES)), trace=trace)
    _LAST_RESULTS = res
    out = np.zeros((NTOK, D), np.float32)
    for r in res.results:
        out += r["outp"]
    return out.reshape(B, S, D)

